# revision 1
# baseline (speedup 1.0000x reference)
"""ContextualLoss forward on 8 trn2 NeuronCores.

Problem: X, Y [4, 256, 64, 64] f32 ->  loss [4] f32
  y_mean[c] = mean_hw(Y);  Xc = X - y_mean; Yc = Y - y_mean
  Xn, Yn: L2-normalized over C per spatial position; S = Xn^T @ Yn  [N, N], N=4096
  d = 1 - S; dmin = row min d; w = exp((1 - d/(dmin+1e-3))/0.1); A = w/rowsum(w)
  loss_b = -log(mean_n max_m A[n, m])

Key algebra used by the kernel (all per row n):
  max_m A[n,:] = wmax[n]/Z[n],
  Z[n]    = sum_m exp(Sh[n,m]*actScale[n] + actBias[n])
  wmax[n] = exp(Smax_raw[n]*actScale[n] + actBias[n])
  where Sh = Xc^T @ Yn  (X centered but NOT normalized; its 1/||Xc|| folds into
  the per-partition (per-row) ACT scale), Smax_raw = row max of Sh,
  g = 1/||Xc||, dminp = 1.001 - Smax_raw*g, actScale = 10*g/dminp,
  actBias = 10 - 10/dminp.

Sharding: 8 cores = 4 batch samples x 2 row-halves of 2048 rows each.
Each core: full Y[b] [256,4096], X rows half [256,2048], computes a
[2048, 4096] similarity block twice on the TensorE (bf16), row-max on the
VectorE straight from PSUM, exp+row-sum on the ScalarE straight from PSUM.
Host combines: loss_b = -log((sum of the two cores' [128,1] outputs)/4096).
"""

import numpy as np

B, C, HW = 4, 256, 4096
HALF = HW // 2
NCORES = 8
NB = HALF // 128      # 16 row blocks per core
MT = HW // 1024       # 4 A/B psum tiles (each [128,1024]) per block
H_INV = 10.0          # 1/h with h = 0.1

_nc_cache = None


def _build():
    import concourse.bass as bass
    import concourse.bacc as bacc
    import concourse.tile as tile
    from concourse import mybir

    f32 = mybir.dt.float32
    f32r = mybir.dt.float32r
    bf16 = mybir.dt.bfloat16
    AF = mybir.ActivationFunctionType
    OP = mybir.AluOpType
    AX = mybir.AxisListType

    nc = bacc.Bacc(None)

    y_dram = nc.dram_tensor("y", [C, HW], f32, kind="ExternalInput")
    x_dram = nc.dram_tensor("xh", [C, HALF], f32, kind="ExternalInput")
    out_dram = nc.dram_tensor("out", [128, 1], f32, kind="ExternalOutput")
    xt_dram = nc.dram_tensor("xt_scratch", [1, HALF], f32)  # transpose bounce

    with tile.TileContext(nc) as tc:
        with (
            tc.tile_pool(name="big", bufs=1) as big,
            tc.tile_pool(name="singles", bufs=1) as singles,
            tc.tile_pool(name="rows", bufs=1) as rows,
            tc.tile_pool(name="stats", bufs=3) as stats,
            tc.tile_pool(name="dumps", bufs=3) as dumps,
        ):
            # ---------------- constants ----------------
            ones_col = singles.tile([128, 1], bf16)
            nc.vector.memset(ones_col, 1.0)
            cm10 = singles.tile([128, 1], f32)
            nc.vector.memset(cm10, -H_INV)
            c1p001 = singles.tile([128, 1], f32)
            nc.vector.memset(c1p001, 1.001)

            # ---------------- load inputs ----------------
            y_sb = [big.tile([128, HW], f32, tag=f"y{i}", name=f"y{i}") for i in range(2)]
            x_sb = [big.tile([128, HALF], f32, tag=f"x{i}", name=f"x{i}") for i in range(2)]
            for i in range(2):
                nc.sync.dma_start(out=y_sb[i], in_=y_dram[128 * i : 128 * (i + 1), :])
            for i in range(2):
                nc.sync.dma_start(out=x_sb[i], in_=x_dram[128 * i : 128 * (i + 1), :])

            # ---------------- spatial mean of Y over positions ----------------
            ysum = [singles.tile([128, 1], f32, tag=f"ysum{i}", name=f"ysum{i}") for i in range(2)]
            for i in range(2):
                nc.vector.reduce_sum(out=ysum[i], in_=y_sb[i], axis=AX.X)
            negmean = [singles.tile([128, 1], f32, tag=f"nm{i}", name=f"nm{i}") for i in range(2)]
            for i in range(2):
                nc.vector.tensor_scalar_mul(out=negmean[i], in0=ysum[i], scalar1=-1.0 / HW)

            # squares of *centered* X/Y: Square(raw + negmean), bf16 (rounded
            # by ACT, so they can legally feed bf16 sum-of-squares matmuls)
            ysq = [big.tile([128, HW], bf16, tag=f"ysq{i}", name=f"ysq{i}") for i in range(2)]
            xsq = [big.tile([128, HALF], bf16, tag=f"xsq{i}", name=f"xsq{i}") for i in range(2)]
            for i in range(2):
                for ch in range(4):
                    sl = slice(ch * 1024, (ch + 1) * 1024)
                    nc.scalar.activation(
                        out=ysq[i][:, sl], in_=y_sb[i][:, sl], func=AF.Square,
                        bias=negmean[i], scale=1.0,
                    )
            for i in range(2):
                for ch in range(2):
                    sl = slice(ch * 1024, (ch + 1) * 1024)
                    nc.scalar.activation(
                        out=xsq[i][:, sl], in_=x_sb[i][:, sl], func=AF.Square,
                        bias=negmean[i], scale=1.0,
                    )

            # centered X in bf16 (matmul lhsT)
            xcb = [big.tile([128, HALF], bf16, tag=f"xcb{i}", name=f"xcb{i}") for i in range(2)]
            for i in range(2):
                nc.scalar.activation(
                    out=xcb[i], in_=x_sb[i], func=AF.Identity, bias=negmean[i], scale=1.0
                )

            invny_row = rows.tile([1, HW], f32)
            invnx_row = rows.tile([1, HALF], f32)

            with tc.tile_pool(name="pspro", bufs=2, space="PSUM") as pspro:
                # per-position sum of squares of the centered features, then
                # 1/norm = exp(-0.5 * ln(ss))
                def inv_norm_row(sq_tiles, out_row, ncols):
                    for chunk in range(ncols // 512):
                        sl = slice(chunk * 512, (chunk + 1) * 512)
                        ss_ps = pspro.tile([1, 512], f32, tag="ss")
                        for i in range(2):
                            nc.tensor.matmul(
                                ss_ps,
                                ones_col,
                                sq_tiles[i][:, sl],
                                start=(i == 0),
                                stop=(i == 1),
                            )
                        lnr = stats.tile([1, 512], f32, tag="lnr")
                        nc.scalar.activation(
                            out=lnr, in_=ss_ps, func=AF.Ln, bias=0.0, scale=1.0
                        )
                        nc.scalar.activation(
                            out=out_row[0:1, sl], in_=lnr, func=AF.Exp, scale=-0.5
                        )

                inv_norm_row(ysq, invny_row, HW)
                inv_norm_row(xsq, invnx_row, HALF)

            # invnX -> [128, 16] (per-partition scalars per row block) via DRAM bounce
            invnx_t = singles.tile([128, NB], f32)
            nc.gpsimd.dma_start(out=xt_dram[:, :], in_=invnx_row)
            nc.gpsimd.dma_start(
                out=invnx_t,
                in_=xt_dram.rearrange("o (j p) -> (o p) j", p=128),
            )

            # broadcast 1/||Yc|| across partitions, then Yn = (Y - mean) * invnY  (bf16)
            invny_b = big.tile([128, HW], f32, tag="invny_b")
            for chunk in range(HW // 512):
                sl = slice(chunk * 512, (chunk + 1) * 512)
                nc.gpsimd.partition_broadcast(invny_b[:, sl], invny_row[0:1, sl])
            yn = [big.tile([128, HW], bf16, tag=f"yn{i}", name=f"yn{i}") for i in range(2)]
            for i in range(2):
                for chunk in range(HW // 512):
                    sl = slice(chunk * 512, (chunk + 1) * 512)
                    nc.vector.scalar_tensor_tensor(
                        out=yn[i][:, sl],
                        in0=y_sb[i][:, sl],
                        scalar=negmean[i],
                        in1=invny_b[:, sl],
                        op0=OP.add,
                        op1=OP.mult,
                    )

            # ---------------- main loop over 16 row blocks ----------------
            wmaxs = singles.tile([128, NB], f32)
            zall = singles.tile([128, NB * MT], f32)

            with (
                tc.tile_pool(name="psA", bufs=2, space="PSUM") as psA,
                tc.tile_pool(name="psB", bufs=2, space="PSUM") as psB,
            ):
                for nb in range(NB):
                    nsl = slice(nb * 128, (nb + 1) * 128)
                    g_col = invnx_t[:, nb : nb + 1]

                    # ---- pass A: row max of Sh ----
                    mx4 = stats.tile([128, MT], f32, tag="mx4")
                    for j in range(MT):
                        pa = psA.tile([128, 1024], f32, tag="pa")
                        for jj in range(2):
                            msl = slice(j * 1024 + jj * 512, j * 1024 + (jj + 1) * 512)
                            osl = slice(jj * 512, (jj + 1) * 512)
                            nc.tensor.matmul(
                                pa[:, osl], xcb[0][:, nsl], yn[0][:, msl],
                                start=True, stop=False,
                            )
                            nc.tensor.matmul(
                                pa[:, osl], xcb[1][:, nsl], yn[1][:, msl],
                                start=False, stop=True,
                            )
                        nc.vector.reduce_max(out=mx4[:, j : j + 1], in_=pa, axis=AX.X)
                    smax = stats.tile([128, 1], f32, tag="smax")
                    nc.vector.reduce_max(out=smax, in_=mx4, axis=AX.X)

                    # ---- per-row exp scale/bias ----
                    # negdminp = smax*g - 1.001 ;  r = 1/negdminp = -1/dminp
                    # actScale = (r*g)*(-10) ;  actBias = 10*r + 10
                    ndm = stats.tile([128, 1], f32, tag="ndm")
                    nc.vector.scalar_tensor_tensor(
                        out=ndm, in0=smax, scalar=g_col, in1=c1p001,
                        op0=OP.mult, op1=OP.subtract,
                    )
                    rr = stats.tile([128, 1], f32, tag="rr")
                    nc.vector.reciprocal(out=rr, in_=ndm)
                    act_scale = stats.tile([128, 1], f32, tag="asc")
                    nc.vector.scalar_tensor_tensor(
                        out=act_scale, in0=rr, scalar=g_col, in1=cm10,
                        op0=OP.mult, op1=OP.mult,
                    )
                    act_bias = stats.tile([128, 1], f32, tag="abi")
                    nc.vector.tensor_scalar(
                        out=act_bias, in0=rr, scalar1=H_INV, scalar2=H_INV,
                        op0=OP.mult, op1=OP.add,
                    )
                    nc.scalar.activation(
                        out=wmaxs[:, nb : nb + 1], in_=smax, func=AF.Exp,
                        bias=act_bias, scale=act_scale,
                    )

                    # ---- pass B: Z = row sum of exp(Sh*scale + bias) ----
                    for j in range(MT):
                        pb = psB.tile([128, 1024], f32, tag="pb")
                        for jj in range(2):
                            msl = slice(j * 1024 + jj * 512, j * 1024 + (jj + 1) * 512)
                            osl = slice(jj * 512, (jj + 1) * 512)
                            nc.tensor.matmul(
                                pb[:, osl], xcb[0][:, nsl], yn[0][:, msl],
                                start=True, stop=False,
                            )
                            nc.tensor.matmul(
                                pb[:, osl], xcb[1][:, nsl], yn[1][:, msl],
                                start=False, stop=True,
                            )
                        dump = dumps.tile([128, 1024], bf16, tag="dump")
                        nc.scalar.activation(
                            out=dump, in_=pb, func=AF.Exp,
                            bias=act_bias, scale=act_scale,
                            accum_out=zall[:, nb * MT + j : nb * MT + j + 1],
                        )

            # ---------------- epilogue: sum_n wmax/Z ----------------
            zs = singles.tile([128, NB], f32)
            nc.vector.reduce_sum(
                out=zs, in_=zall.rearrange("p (nb mt) -> p nb mt", mt=MT), axis=AX.X
            )
            rz = singles.tile([128, NB], f32)
            nc.vector.reciprocal(out=rz, in_=zs)
            vals = singles.tile([128, NB], f32)
            nc.vector.tensor_tensor(out=vals, in0=wmaxs, in1=rz, op=OP.mult)
            acc = singles.tile([128, 1], f32)
            nc.vector.reduce_sum(out=acc, in_=vals, axis=AX.X)
            nc.gpsimd.dma_start(out=out_dram[:, :], in_=acc)

    nc.finalize()
    return nc


def _get_nc():
    global _nc_cache
    if _nc_cache is None:
        _nc_cache = _build()
    return _nc_cache


def run_cores(inputs, **kwargs):
    """Run the 8-core SPMD kernel; returns (loss[4], BassKernelResults)."""
    from concourse.bass_utils import run_bass_kernel_spmd

    nc = _get_nc()
    X = np.asarray(inputs["X_features"], dtype=np.float32).reshape(B, C, HW)
    Y = np.asarray(inputs["Y_features"], dtype=np.float32).reshape(B, C, HW)
    in_maps = []
    for core in range(NCORES):
        b, h = divmod(core, 2)
        in_maps.append(
            {
                "y": np.ascontiguousarray(Y[b]),
                "xh": np.ascontiguousarray(X[b, :, h * HALF : (h + 1) * HALF]),
            }
        )
    res = run_bass_kernel_spmd(nc, in_maps, core_ids=list(range(NCORES)), **kwargs)
    acc = np.stack(
        [res.results[i]["out"].reshape(-1).astype(np.float64) for i in range(NCORES)]
    )  # [8, 128]
    cx = acc.reshape(B, 2 * 128).sum(axis=1) / HW
    loss = (-np.log(cx)).astype(np.float32)
    return loss, res


def kernel(**inputs):
    return run_cores(inputs)[0]

